# revision 19
# baseline (speedup 1.0000x reference)
"""DeepSets (MLP + ragged segment-mean) Trainium2 Bass kernel.

Full inputs in / full outputs out. Internally: data-parallel over sets --
tokens are sharded by contiguous whole-segment ranges across 8 NeuronCores
(balanced by token count), the tiny MLP weights are replicated, and the
segment-mean is fully local per core.

Per-core device pipeline (feature-major, x pre-transposed on host):
  L1  : psum_h1[dh,t] = W1.T @ xT           (TensorE, fp32r, weights stationary)
  relu: h1 = relu(psum_h1 + b1)             (ScalarE activation, fused bias)
  L2  : psum_h2[f,t] = W2.T @ h1            (TensorE, fp32r, 2-chunk accumulate)
  relu: h2 = relu(psum_h2 + b2)             (VectorE tensor_scalar, fused bias)
  scan: cum[f,t] = running sum of h2 cols   (VectorE tensor_tensor_scan, fp32 state)
  gath: g[slot] = cum at segment-end cols   (GpSimd ap_gather, host-built indices)
  diff: totals = g[1:] - g[:-1]             (VectorE)
  out : transpose 128-slot tiles (TensorE) -> scale by 1/count (VectorE) -> DMA
"""

import math
from contextlib import ExitStack

import numpy as np

import concourse.bass as bass
import concourse.tile as tile
from concourse import bacc, mybir
from concourse.bass_utils import run_bass_kernel_spmd

N_CORES = 8
D_IN, D_H, D_OUT = 128, 256, 128
WIN = 2048  # tokens per scan/gather window
ITER = 1024  # tokens per MLP pipeline iteration (2-bank psum tiles)
MMT = 512  # max fp32 matmul moving dim = one psum bank

F32 = mybir.dt.float32
F32R = mybir.dt.float32r
I16 = mybir.dt.int16
RELU = mybir.ActivationFunctionType.Relu
ADD = mybir.AluOpType.add
SUB = mybir.AluOpType.subtract
MULT = mybir.AluOpType.mult
MAX = mybir.AluOpType.max


def _build_program(t_pad: int, spw: int, n_tr: int, reps: int = 1, mode: str = "full"):
    """Build the single-core SPMD program for t_pad tokens per core.

    spw: gather slots per window (multiple of 16)
    n_tr: number of 128-slot output tiles (out rows = n_tr*128)
    reps: execute the whole pipeline this many times (timing use only)
    mode: "full" | "mlp" (skip scan+gather) | "scan" (skip gather) --
          ablation timing only; non-full modes give wrong results
    """
    n_win = t_pad // WIN
    spw16 = spw // 16
    # idx blocks padded to 8 int16 columns (16B) so each window's slice is
    # cacheline-aligned -- GpSimd misreads 2-byte-misaligned idx slices
    idxp = ((spw16 + 7) // 8) * 8
    g_len = n_tr * 128

    nc = bacc.Bacc(
        "TRN2", target_bir_lowering=False, debug=False, num_devices=N_CORES
    )
    xT = nc.dram_tensor("xT", [D_IN, t_pad], F32R, kind="ExternalInput").ap()
    w1 = nc.dram_tensor("w1", [D_IN, D_H], F32R, kind="ExternalInput").ap()
    # w2 packed on host: [:, 0:128] = W2[0:128,:], [:, 128:256] = W2[128:256,:]
    w2 = nc.dram_tensor("w2", [128, 2 * D_OUT], F32R, kind="ExternalInput").ap()
    b1 = nc.dram_tensor("b1", [128, 2], F32, kind="ExternalInput").ap()
    b2 = nc.dram_tensor("b2", [128, 1], F32, kind="ExternalInput").ap()
    eye = nc.dram_tensor("eye", [128, 128], F32, kind="ExternalInput").ap()
    gidx = nc.dram_tensor("gidx", [128, n_win * idxp], I16, kind="ExternalInput").ap()
    invc = nc.dram_tensor("invc", [128, n_tr], F32, kind="ExternalInput").ap()
    out = nc.dram_tensor("out", [g_len, D_OUT], F32, kind="ExternalOutput").ap()

    with tile.TileContext(nc) as tc, ExitStack() as ctx:
        singles = ctx.enter_context(tc.tile_pool(name="singles", bufs=1))
        xin = ctx.enter_context(tc.tile_pool(name="xin", bufs=3))
        h1sb = ctx.enter_context(tc.tile_pool(name="h1sb", bufs=2))
        h2winp = ctx.enter_context(tc.tile_pool(name="h2win", bufs=2))
        winp = ctx.enter_context(tc.tile_pool(name="winp", bufs=2))
        gp = ctx.enter_context(tc.tile_pool(name="gp", bufs=1))
        outp = ctx.enter_context(tc.tile_pool(name="outp", bufs=2))
        # 8 psum banks total: ps1 holds h1a and h2 (2 slots x 2 banks),
        # ps2 holds h1b (2 slots x 2 banks). Alloc order makes h1a always
        # slot 0 and h2 always slot 1 of ps1.
        ps1 = ctx.enter_context(tc.tile_pool(name="ps1", bufs=2, space="PSUM"))
        ps2 = ctx.enter_context(tc.tile_pool(name="ps2", bufs=2, space="PSUM"))

        w1s = singles.tile([128, D_H], F32R)
        nc.sync.dma_start(out=w1s[:], in_=w1[:])
        w2s = singles.tile([128, 2 * D_OUT], F32R)
        nc.sync.dma_start(out=w2s[:], in_=w2[:])
        b1s = singles.tile([128, 2], F32)
        nc.sync.dma_start(out=b1s[:], in_=b1[:])
        b2s = singles.tile([128, 1], F32)
        nc.sync.dma_start(out=b2s[:], in_=b2[:])
        eyes = singles.tile([128, 128], F32)
        nc.sync.dma_start(out=eyes[:], in_=eye[:])
        gis = singles.tile([128, n_win * idxp], I16)
        nc.sync.dma_start(out=gis[:], in_=gidx[:])
        ics = singles.tile([128, n_tr], F32)
        nc.sync.dma_start(out=ics[:], in_=invc[:])
        ones = singles.tile([128, WIN], F32)
        nc.vector.memset(ones[:], 1.0)

        gpt = gp.tile([128, 1 + g_len], F32, tag="gpad")
        nc.gpsimd.memset(gpt[:], 0.0)

        for _rep in range(reps):
          # timing-only outer repetition; each rep rewrites the same output
          prev_win = None
          for w in range(n_win):
            h2w = h2winp.tile([128, WIN], F32, tag="h2w")
            # one big input DMA per window: 8KB per partition amortizes the
            # per-descriptor SDMA overhead (2KB chunks measured ~58 GB/s)
            xw = xin.tile([128, WIN], F32R, tag="xw")
            nc.sync.dma_start(out=xw[:], in_=xT[:, w * WIN : (w + 1) * WIN])
            if mode == "dma":
                nc.vector.tensor_copy(out=gpt[:, 0:1], in_=xw[:, 0:1].bitcast(F32))
                continue
            for it in range(WIN // ITER):
                xt = xw[:, it * ITER : (it + 1) * ITER]
                h1a_ps = ps1.tile([128, ITER], F32, tag="ps")
                h1b_ps = ps2.tile([128, ITER], F32, tag="psb")
                for j in (0, 1):
                    sl = slice(j * MMT, (j + 1) * MMT)
                    nc.tensor.matmul(
                        h1a_ps[:, sl], w1s[:, 0:128], xt[:, sl], start=True, stop=True
                    )
                    nc.tensor.matmul(
                        h1b_ps[:, sl], w1s[:, 128:256], xt[:, sl], start=True, stop=True
                    )
                if mode == "mm":
                    # consume psum minimally; skip evacuation + L2
                    nc.vector.tensor_copy(out=gpt[:, 0:1], in_=h1a_ps[:, 0:1])
                    nc.vector.tensor_copy(out=gpt[:, 0:1], in_=h1b_ps[:, 0:1])
                    continue
                h1a = h1sb.tile([128, ITER], F32R, tag="h1a")
                h1b = h1sb.tile([128, ITER], F32R, tag="h1b")
                nc.scalar.activation(h1a[:], h1a_ps[:], RELU, bias=b1s[:, 0:1])
                nc.scalar.activation(h1b[:], h1b_ps[:], RELU, bias=b1s[:, 1:2])
                h2_ps = ps1.tile([128, ITER], F32, tag="ps")
                for j in (0, 1):
                    sl = slice(j * MMT, (j + 1) * MMT)
                    nc.tensor.matmul(
                        h2_ps[:, sl], w2s[:, 0:128], h1a[:, sl], start=True, stop=False
                    )
                    nc.tensor.matmul(
                        h2_ps[:, sl], w2s[:, 128:256], h1b[:, sl], start=False, stop=True
                    )
                # h2 = relu(psum + b2), written into the window-wide buffer
                nc.vector.tensor_scalar(
                    out=h2w[:, it * ITER : (it + 1) * ITER],
                    in0=h2_ps[:],
                    scalar1=b2s[:, 0:1],
                    scalar2=0.0,
                    op0=ADD,
                    op1=MAX,
                )
            if mode in ("dma", "mm"):
                continue
            if mode == "mlp":
                # keep a consumer for h2w so the pipeline shape is preserved
                nc.vector.tensor_copy(
                    out=gpt[:, 0:1], in_=h2w[:, WIN - 1 : WIN]
                )
                continue
            win = winp.tile([128, 1 + WIN], F32, tag="win")
            if w == 0:
                nc.vector.memset(win[:, 0:1], 0.0)
            else:
                nc.vector.tensor_copy(out=win[:, 0:1], in_=prev_win[:, WIN : WIN + 1])
            nc.vector.tensor_tensor_scan(
                out=win[:, 1 : 1 + WIN],
                data0=ones[:],
                data1=h2w[:],
                initial=win[:, 0:1],
                op0=MULT,
                op1=ADD,
            )
            if mode != "scan":
                nc.gpsimd.ap_gather(
                    out_ap=gpt[:, 1 + w * spw : 1 + (w + 1) * spw],
                    in_ap=win[:],
                    idxs_ap=gis[:, w * idxp : w * idxp + spw16],
                    channels=128,
                    num_elems=WIN + 1,
                    d=1,
                    num_idxs=spw,
                )
            prev_win = win

          totals = gp.tile([128, g_len], F32, tag="totals")
          nc.vector.tensor_tensor(
              out=totals[:], in0=gpt[:, 1 : 1 + g_len], in1=gpt[:, 0:g_len], op=SUB
          )
          for t in range(n_tr):
            pst = ps1.tile([128, 128], F32, tag="ps")
            nc.tensor.transpose(pst[:], totals[:, t * 128 : (t + 1) * 128], eyes[:])
            ot = outp.tile([128, 128], F32, tag="ot")
            nc.vector.tensor_scalar_mul(ot[:], pst[:], ics[:, t : t + 1])
            nc.sync.dma_start(out=out[t * 128 : (t + 1) * 128, :], in_=ot[:])

    nc.compile()
    return nc


def _prepare(x, segment_ids, num_segments):
    """Host-side sharding + gather-index construction. Returns per-core
    metadata and the program size parameters."""
    T_total = x.shape[0]
    n_seg = int(num_segments)
    seg = np.asarray(segment_ids).astype(np.int64)
    counts = np.bincount(seg, minlength=n_seg).astype(np.int64)
    assert counts.max() < WIN, "segment longer than scan window unsupported"
    cum = np.cumsum(counts)

    # whole-segment split balanced by token count
    split = [0]
    for c in range(1, N_CORES):
        target = c * T_total / N_CORES
        s = int(np.searchsorted(cum, target))
        # choose s or s+1, whichever boundary token count is closer
        if s + 1 < n_seg and abs(cum[s] - target) < abs(
            (cum[s - 1] if s > 0 else 0) - target
        ):
            s = s + 1
        s = max(split[-1], min(s, n_seg))
        split.append(s)
    split.append(n_seg)

    cores = []
    max_tok = 1
    for c in range(N_CORES):
        s0, s1 = split[c], split[c + 1]
        t0 = int(cum[s0 - 1]) if s0 > 0 else 0
        t1 = int(cum[s1 - 1]) if s1 > 0 else 0
        cores.append({"s0": s0, "s1": s1, "t0": t0, "t1": t1})
        max_tok = max(max_tok, t1 - t0)

    t_pad = int(math.ceil(max_tok / WIN) * WIN)
    n_win = t_pad // WIN

    # per-core per-window segment-end indices
    max_ends = 1
    for core in cores:
        s0, s1, t0 = core["s0"], core["s1"], core["t0"]
        ends = cum[s0:s1] - 1 - t0  # local end col per segment; may be -1
        win_of = np.maximum(ends, 0) // WIN
        idx_rel = ends - win_of * WIN + 1  # in [0, WIN]
        core["win_of"] = win_of
        core["idx_rel"] = idx_rel
        if len(ends):
            bc = np.bincount(win_of, minlength=n_win)
            max_ends = max(max_ends, int(bc.max()))

    spw = int(math.ceil(max_ends / 16) * 16)
    n_tr = int(math.ceil(n_win * spw / 128))

    for core in cores:
        s0, s1 = core["s0"], core["s1"]
        n_loc = s1 - s0
        slot_of = np.zeros(n_loc, dtype=np.int64)
        idx_full = np.zeros(n_win * spw, dtype=np.int16)
        pos = np.zeros(n_win, dtype=np.int64)
        # fill window-by-window in segment order
        for j in range(n_loc):
            w = int(core["win_of"][j])
            k = int(pos[w])
            assert k < spw
            idx_full[w * spw + k] = core["idx_rel"][j]
            slot_of[j] = w * spw + k
            pos[w] = k + 1
        # pad each window by repeating its last real index (0 if none)
        for w in range(n_win):
            k = int(pos[w])
            last = idx_full[w * spw + k - 1] if k > 0 else np.int16(0)
            idx_full[w * spw + k : (w + 1) * spw] = last
        core["slot_of"] = slot_of
        # wrap for ap_gather: unwrapped[j] = idxs[j % 16, j // 16] per window,
        # each block padded to a 16B-aligned width
        idxp = ((spw // 16 + 7) // 8) * 8
        blocks = []
        for w in range(n_win):
            arr = idx_full[w * spw : (w + 1) * spw]
            blk = np.zeros((16, idxp), dtype=np.int16)
            blk[:, : spw // 16] = arr.reshape(spw // 16, 16).T
            blocks.append(blk)
        gidx16 = np.concatenate(blocks, axis=1)  # [16, n_win * idxp]
        core["gidx"] = np.tile(gidx16, (8, 1)).astype(np.int16)  # [128, ...]
        inv_slot = np.zeros(n_tr * 128, dtype=np.float32)
        inv_slot[slot_of] = 1.0 / np.maximum(counts[s0:s1], 1)
        core["invc"] = np.ascontiguousarray(
            inv_slot.reshape(n_tr, 128).T
        )  # [128, n_tr]

    return cores, t_pad, spw, n_tr


_PROGRAM_CACHE = {}


def kernel(x, segment_ids, num_segments, W1, b1, W2, b2):
    x = np.ascontiguousarray(np.asarray(x, dtype=np.float32))
    W1 = np.asarray(W1, dtype=np.float32)
    b1 = np.asarray(b1, dtype=np.float32)
    W2 = np.asarray(W2, dtype=np.float32)
    b2 = np.asarray(b2, dtype=np.float32)
    n_seg = int(num_segments)

    cores, t_pad, spw, n_tr = _prepare(x, segment_ids, num_segments)

    key = (t_pad, spw, n_tr)
    if key not in _PROGRAM_CACHE:
        _PROGRAM_CACHE[key] = _build_program(t_pad, spw, n_tr)
    nc = _PROGRAM_CACHE[key]

    w2_np = np.ascontiguousarray(np.concatenate([W2[:128, :], W2[128:, :]], axis=1))
    b1_np = np.ascontiguousarray(np.stack([b1[:128], b1[128:]], axis=1))
    b2_np = np.ascontiguousarray(b2[:, None])
    eye_np = np.eye(128, dtype=np.float32)

    in_maps = []
    for core in cores:
        t0, t1 = core["t0"], core["t1"]
        xT_c = np.zeros((D_IN, t_pad), dtype=np.float32)
        xT_c[:, : t1 - t0] = x[t0:t1].T
        in_maps.append(
            {
                "xT": xT_c,
                "w1": W1,
                "w2": w2_np,
                "b1": b1_np,
                "b2": b2_np,
                "eye": eye_np,
                "gidx": core["gidx"],
                "invc": core["invc"],
            }
        )

    res = run_bass_kernel_spmd(nc, in_maps, list(range(N_CORES)))

    out_full = np.zeros((n_seg, D_OUT), dtype=np.float32)
    for c, core in enumerate(cores):
        s0, s1 = core["s0"], core["s1"]
        if s1 > s0:
            out_full[s0:s1] = res.results[c]["out"][core["slot_of"]]
    return out_full


# revision 39
# speedup vs baseline: 1.6124x; 1.6124x over previous
"""DeepSets (MLP + ragged segment-mean) Trainium2 Bass kernel.

Full inputs in / full outputs out. Internally: data-parallel over sets --
tokens are sharded by contiguous whole-segment ranges across 8 NeuronCores
(balanced by token count), the tiny MLP weights are replicated, and the
segment-mean is fully local per core.

Per-core device pipeline (feature-major, x pre-transposed on host):
  L1  : psum_h1[dh,t] = W1.T @ xT           (TensorE, fp32r, weights stationary)
  relu: h1 = relu(psum_h1 + b1)             (ScalarE activation, fused bias)
  L2  : psum_h2[f,t] = W2.T @ h1            (TensorE, fp32r, 2-chunk accumulate)
  relu: h2 = relu(psum_h2 + b2)             (VectorE tensor_scalar, fused bias)
  scan: cum[f,t] = running sum of h2 cols   (VectorE tensor_tensor_scan, fp32 state)
  gath: g[slot] = cum at segment-end cols   (GpSimd ap_gather, host-built indices)
  diff: totals = g[1:] - g[:-1]             (VectorE)
  out : transpose 128-slot tiles (TensorE) -> scale by 1/count (VectorE) -> DMA
"""

import math
from contextlib import ExitStack

import numpy as np

import concourse.bass as bass
import concourse.tile as tile
from concourse import bacc, mybir
from concourse.bass_utils import run_bass_kernel_spmd

N_CORES = 8
D_IN, D_H, D_OUT = 128, 256, 128
WIN = 2048  # tokens per scan/gather window
ITER = 512  # tokens per MLP pipeline iteration
MMT = 512  # max fp32 matmul moving dim = one psum bank
SBUF_BUFS = 3  # double/triple buffering depth for streaming sbuf pools
PS_BUFS = (2, 2, 2)  # psum pool depths for (h1a, h1b, h2); total banks <= 8
H2_EVAC = "act_psum"  # "act_psum": ACT relu in place, scan reads psum
#                       "dve_sbuf": DVE relu -> sbuf buffer, window-wide scan
H1_SPLIT = True  # split h1 evacuations between ACT and DVE

F32 = mybir.dt.float32
F32R = mybir.dt.float32r
I16 = mybir.dt.int16
RELU = mybir.ActivationFunctionType.Relu
ADD = mybir.AluOpType.add
SUB = mybir.AluOpType.subtract
MULT = mybir.AluOpType.mult
MAX = mybir.AluOpType.max


def _build_program(t_pad: int, spw: int, n_tr: int, reps: int = 1, mode: str = "full"):
    """Build the single-core SPMD program for t_pad tokens per core.

    spw: gather slots per window (multiple of 16)
    n_tr: number of 128-slot output tiles (out rows = n_tr*128)
    reps: execute the whole pipeline this many times (timing use only)
    mode: "full" | "mlp" (skip scan+gather) | "scan" (skip gather) --
          ablation timing only; non-full modes give wrong results
    """
    n_win = t_pad // WIN
    spw16 = spw // 16
    # idx blocks padded to 8 int16 columns (16B) so each window's slice is
    # cacheline-aligned -- GpSimd misreads 2-byte-misaligned idx slices
    idxp = ((spw16 + 7) // 8) * 8
    g_len = n_tr * 128

    nc = bacc.Bacc(
        "TRN2", target_bir_lowering=False, debug=False, num_devices=N_CORES
    )
    xT = nc.dram_tensor("xT", [D_IN, t_pad], F32R, kind="ExternalInput").ap()
    w1 = nc.dram_tensor("w1", [D_IN, D_H], F32R, kind="ExternalInput").ap()
    # w2 packed on host: [:, 0:128] = W2[0:128,:], [:, 128:256] = W2[128:256,:]
    w2 = nc.dram_tensor("w2", [128, 2 * D_OUT], F32R, kind="ExternalInput").ap()
    b1 = nc.dram_tensor("b1", [128, 2], F32, kind="ExternalInput").ap()
    b2 = nc.dram_tensor("b2", [128, 1], F32, kind="ExternalInput").ap()
    eye = nc.dram_tensor("eye", [128, 128], F32, kind="ExternalInput").ap()
    gidx = nc.dram_tensor("gidx", [128, n_win * idxp], I16, kind="ExternalInput").ap()
    invc = nc.dram_tensor("invc", [128, n_tr], F32, kind="ExternalInput").ap()
    out = nc.dram_tensor("out", [g_len, D_OUT], F32, kind="ExternalOutput").ap()

    with tile.TileContext(nc) as tc, ExitStack() as ctx:
        singles = ctx.enter_context(tc.tile_pool(name="singles", bufs=1))
        xin = ctx.enter_context(tc.tile_pool(name="xin", bufs=SBUF_BUFS))
        h1sb = ctx.enter_context(tc.tile_pool(name="h1sb", bufs=SBUF_BUFS))
        winp = ctx.enter_context(tc.tile_pool(name="winp", bufs=2))
        gp = ctx.enter_context(tc.tile_pool(name="gp", bufs=1))
        outp = ctx.enter_context(tc.tile_pool(name="outp", bufs=2))
        if H2_EVAC == "dve_sbuf":
            h2winp = ctx.enter_context(tc.tile_pool(name="h2win", bufs=2))
        ps1 = ctx.enter_context(
            tc.tile_pool(name="ps1", bufs=PS_BUFS[0], space="PSUM")
        )
        ps2 = ctx.enter_context(
            tc.tile_pool(name="ps2", bufs=PS_BUFS[1], space="PSUM")
        )
        ps3 = ctx.enter_context(
            tc.tile_pool(name="ps3", bufs=PS_BUFS[2], space="PSUM")
        )
        pst_pool = ctx.enter_context(tc.tile_pool(name="pst", bufs=1, space="PSUM"))
        totp = ctx.enter_context(tc.tile_pool(name="totp", bufs=2))

        w1s = singles.tile([128, D_H], F32R)
        nc.sync.dma_start(out=w1s[:], in_=w1[:])
        w2s = singles.tile([128, 2 * D_OUT], F32R)
        nc.sync.dma_start(out=w2s[:], in_=w2[:])
        b1s = singles.tile([128, 2], F32)
        nc.sync.dma_start(out=b1s[:], in_=b1[:])
        b2s = singles.tile([128, 1], F32)
        nc.sync.dma_start(out=b2s[:], in_=b2[:])
        eyes = singles.tile([128, 128], F32)
        nc.sync.dma_start(out=eyes[:], in_=eye[:])
        gis = singles.tile([128, n_win * idxp], I16)
        nc.sync.dma_start(out=gis[:], in_=gidx[:])
        ics = singles.tile([128, n_tr], F32)
        nc.sync.dma_start(out=ics[:], in_=invc[:])
        ones = singles.tile([128, WIN], F32)
        nc.vector.memset(ones[:], 1.0)
        wsum = singles.tile([128, n_win], F32)

        gpt = gp.tile([128, 1 + g_len], F32, tag="gpad")
        nc.gpsimd.memset(gpt[:], 0.0)

        def emit_tile_epilogue(t):
            """Difference 128 slots, fix window-boundary slots, transpose to
            segment-major, scale by 1/count, DMA out."""
            tt = totp.tile([128, 128], F32, tag="tot")
            nc.vector.tensor_tensor(
                out=tt[:],
                in0=gpt[:, 1 + t * 128 : 1 + (t + 1) * 128],
                in1=gpt[:, t * 128 : (t + 1) * 128],
                op=SUB,
            )
            wb = (t * 128 + spw - 1) // spw  # first window boundary in range
            while wb * spw < (t + 1) * 128 and wb < n_win:
                if wb >= 1:
                    col = wb * spw - t * 128
                    nc.vector.tensor_tensor(
                        out=tt[:, col : col + 1],
                        in0=tt[:, col : col + 1],
                        in1=wsum[:, wb - 1 : wb],
                        op=ADD,
                    )
                wb += 1
            pst = pst_pool.tile([128, 128], F32, tag="pst")
            nc.tensor.transpose(pst[:], tt[:], eyes[:])
            ot = outp.tile([128, 128], F32, tag="ot")
            nc.vector.tensor_scalar_mul(ot[:], pst[:], ics[:, t : t + 1])
            nc.sync.dma_start(out=out[t * 128 : (t + 1) * 128, :], in_=ot[:])

        for _rep in range(reps):
          # timing-only outer repetition; each rep rewrites the same output
          prev_win = None
          done_tiles = 0
          for w in range(n_win):
            # one big input DMA per window: 8KB per partition amortizes the
            # per-descriptor SDMA overhead (2KB chunks measured ~58 GB/s)
            xw = xin.tile([128, WIN], F32R, tag="xw")
            nc.sync.dma_start(out=xw[:], in_=xT[:, w * WIN : (w + 1) * WIN])
            if mode == "dma":
                nc.vector.tensor_copy(out=gpt[:, 0:1], in_=xw[:, 0:1].bitcast(F32))
                continue
            # per-window local cumsum (reset to 0): scans of different windows
            # are independent; cross-window carry is restored by the
            # boundary-slot fixup after differencing
            win = winp.tile([128, 1 + WIN], F32, tag="win")
            if mode == "full":
                nc.vector.memset(win[:, 0:1], 0.0)
            if H2_EVAC == "dve_sbuf":
                h2w = h2winp.tile([128, WIN], F32, tag="h2w")
            for it in range(WIN // ITER):
                xt = xw[:, it * ITER : (it + 1) * ITER]
                h1a_ps = ps1.tile([128, ITER], F32, tag="h1a_ps")
                h1b_ps = ps2.tile([128, ITER], F32, tag="h1b_ps")
                for j in range(ITER // MMT):
                    sl = slice(j * MMT, (j + 1) * MMT)
                    nc.tensor.matmul(
                        h1a_ps[:, sl], w1s[:, 0:128], xt[:, sl], start=True, stop=True
                    )
                    nc.tensor.matmul(
                        h1b_ps[:, sl], w1s[:, 128:256], xt[:, sl], start=True, stop=True
                    )
                if mode == "mm":
                    # consume psum minimally; skip evacuation + L2
                    nc.vector.tensor_copy(out=gpt[:, 0:1], in_=h1a_ps[:, 0:1])
                    nc.vector.tensor_copy(out=gpt[:, 0:1], in_=h1b_ps[:, 0:1])
                    continue
                h1a = h1sb.tile([128, ITER], F32R, tag="h1a")
                h1b = h1sb.tile([128, ITER], F32R, tag="h1b")

                # relu(psum + b1) evacuations, split ACT/DVE for balance
                def evac(dst, src, bias_ap, on_act):
                    if on_act:
                        nc.scalar.activation(dst[:], src[:], RELU, bias=bias_ap)
                    else:
                        nc.vector.tensor_scalar(
                            out=dst[:],
                            in0=src[:],
                            scalar1=bias_ap,
                            scalar2=0.0,
                            op0=ADD,
                            op1=MAX,
                        )

                if H1_SPLIT:
                    act_a = (w + it) % 2 == 0
                    evac(h1a, h1a_ps, b1s[:, 0:1], on_act=act_a)
                    evac(h1b, h1b_ps, b1s[:, 1:2], on_act=not act_a)
                else:
                    evac(h1a, h1a_ps, b1s[:, 0:1], on_act=True)
                    evac(h1b, h1b_ps, b1s[:, 1:2], on_act=True)
                h2_ps = ps3.tile([128, ITER], F32, tag="h2_ps")
                for j in range(ITER // MMT):
                    sl = slice(j * MMT, (j + 1) * MMT)
                    nc.tensor.matmul(
                        h2_ps[:, sl], w2s[:, 0:128], h1a[:, sl], start=True, stop=False
                    )
                    nc.tensor.matmul(
                        h2_ps[:, sl], w2s[:, 128:256], h1b[:, sl], start=False, stop=True
                    )
                if H2_EVAC == "act_psum":
                    # h2 = relu(psum + b2) on ACT, in place in psum
                    nc.scalar.activation(h2_ps[:], h2_ps[:], RELU, bias=b2s[:, 0:1])
                    if mode == "mlp":
                        nc.vector.tensor_copy(out=gpt[:, 0:1], in_=h2_ps[:, 0:1])
                        continue
                    # chunk cumsum chained via initial; scan reads psum
                    nc.vector.tensor_tensor_scan(
                        out=win[:, 1 + it * ITER : 1 + (it + 1) * ITER],
                        data0=ones[:, 0:ITER],
                        data1=h2_ps[:],
                        initial=win[:, it * ITER : it * ITER + 1],
                        op0=MULT,
                        op1=ADD,
                    )
                else:
                    # h2 = relu(psum + b2) on DVE into the window buffer
                    nc.vector.tensor_scalar(
                        out=h2w[:, it * ITER : (it + 1) * ITER],
                        in0=h2_ps[:],
                        scalar1=b2s[:, 0:1],
                        scalar2=0.0,
                        op0=ADD,
                        op1=MAX,
                    )
            if mode in ("dma", "mm", "mlp"):
                continue
            if H2_EVAC == "dve_sbuf":
                nc.vector.tensor_tensor_scan(
                    out=win[:, 1 : 1 + WIN],
                    data0=ones[:],
                    data1=h2w[:],
                    initial=win[:, 0:1],
                    op0=MULT,
                    op1=ADD,
                )
            # keep this window's total for the cross-window boundary fixup
            nc.vector.tensor_copy(out=wsum[:, w : w + 1], in_=win[:, WIN : WIN + 1])
            if mode != "scan":
                nc.gpsimd.ap_gather(
                    out_ap=gpt[:, 1 + w * spw : 1 + (w + 1) * spw],
                    in_ap=win[:],
                    idxs_ap=gis[:, w * idxp : w * idxp + spw16],
                    channels=128,
                    num_elems=WIN + 1,
                    d=1,
                    num_idxs=spw,
                )
                # emit output tiles whose slot range is now fully gathered
                avail = n_tr if w == n_win - 1 else ((w + 1) * spw) // 128
                while done_tiles < min(avail, n_tr):
                    emit_tile_epilogue(done_tiles)
                    done_tiles += 1
            prev_win = win

    nc.compile()
    return nc


def _prepare(x, segment_ids, num_segments):
    """Host-side sharding + gather-index construction. Returns per-core
    metadata and the program size parameters."""
    T_total = x.shape[0]
    n_seg = int(num_segments)
    seg = np.asarray(segment_ids).astype(np.int64)
    counts = np.bincount(seg, minlength=n_seg).astype(np.int64)
    assert counts.max() < WIN, "segment longer than scan window unsupported"
    cum = np.cumsum(counts)

    # whole-segment split balanced by token count
    split = [0]
    for c in range(1, N_CORES):
        target = c * T_total / N_CORES
        s = int(np.searchsorted(cum, target))
        # choose s or s+1, whichever boundary token count is closer
        if s + 1 < n_seg and abs(cum[s] - target) < abs(
            (cum[s - 1] if s > 0 else 0) - target
        ):
            s = s + 1
        s = max(split[-1], min(s, n_seg))
        split.append(s)
    split.append(n_seg)

    cores = []
    max_tok = 1
    for c in range(N_CORES):
        s0, s1 = split[c], split[c + 1]
        t0 = int(cum[s0 - 1]) if s0 > 0 else 0
        t1 = int(cum[s1 - 1]) if s1 > 0 else 0
        cores.append({"s0": s0, "s1": s1, "t0": t0, "t1": t1})
        max_tok = max(max_tok, t1 - t0)

    t_pad = int(math.ceil(max_tok / WIN) * WIN)
    n_win = t_pad // WIN

    # per-core per-window segment-end indices
    max_ends = 1
    for core in cores:
        s0, s1, t0 = core["s0"], core["s1"], core["t0"]
        ends = cum[s0:s1] - 1 - t0  # local end col per segment; may be -1
        win_of = np.maximum(ends, 0) // WIN
        idx_rel = ends - win_of * WIN + 1  # in [0, WIN]
        core["win_of"] = win_of
        core["idx_rel"] = idx_rel
        if len(ends):
            bc = np.bincount(win_of, minlength=n_win)
            max_ends = max(max_ends, int(bc.max()))

    spw = int(math.ceil(max_ends / 16) * 16)
    n_tr = int(math.ceil(n_win * spw / 128))

    for core in cores:
        s0, s1 = core["s0"], core["s1"]
        n_loc = s1 - s0
        slot_of = np.zeros(n_loc, dtype=np.int64)
        idx_full = np.zeros(n_win * spw, dtype=np.int16)
        pos = np.zeros(n_win, dtype=np.int64)
        # fill window-by-window in segment order
        for j in range(n_loc):
            w = int(core["win_of"][j])
            k = int(pos[w])
            assert k < spw
            idx_full[w * spw + k] = core["idx_rel"][j]
            slot_of[j] = w * spw + k
            pos[w] = k + 1
        # pad each window by repeating its last real index (0 if none)
        for w in range(n_win):
            k = int(pos[w])
            last = idx_full[w * spw + k - 1] if k > 0 else np.int16(0)
            idx_full[w * spw + k : (w + 1) * spw] = last
        core["slot_of"] = slot_of
        # wrap for ap_gather: unwrapped[j] = idxs[j % 16, j // 16] per window,
        # each block padded to a 16B-aligned width
        idxp = ((spw // 16 + 7) // 8) * 8
        blocks = []
        for w in range(n_win):
            arr = idx_full[w * spw : (w + 1) * spw]
            blk = np.zeros((16, idxp), dtype=np.int16)
            blk[:, : spw // 16] = arr.reshape(spw // 16, 16).T
            blocks.append(blk)
        gidx16 = np.concatenate(blocks, axis=1)  # [16, n_win * idxp]
        core["gidx"] = np.tile(gidx16, (8, 1)).astype(np.int16)  # [128, ...]
        inv_slot = np.zeros(n_tr * 128, dtype=np.float32)
        inv_slot[slot_of] = 1.0 / np.maximum(counts[s0:s1], 1)
        core["invc"] = np.ascontiguousarray(
            inv_slot.reshape(n_tr, 128).T
        )  # [128, n_tr]

    return cores, t_pad, spw, n_tr


_PROGRAM_CACHE = {}


def kernel(x, segment_ids, num_segments, W1, b1, W2, b2):
    x = np.ascontiguousarray(np.asarray(x, dtype=np.float32))
    W1 = np.asarray(W1, dtype=np.float32)
    b1 = np.asarray(b1, dtype=np.float32)
    W2 = np.asarray(W2, dtype=np.float32)
    b2 = np.asarray(b2, dtype=np.float32)
    n_seg = int(num_segments)

    cores, t_pad, spw, n_tr = _prepare(x, segment_ids, num_segments)

    key = (t_pad, spw, n_tr)
    if key not in _PROGRAM_CACHE:
        _PROGRAM_CACHE[key] = _build_program(t_pad, spw, n_tr)
    nc = _PROGRAM_CACHE[key]

    w2_np = np.ascontiguousarray(np.concatenate([W2[:128, :], W2[128:, :]], axis=1))
    b1_np = np.ascontiguousarray(np.stack([b1[:128], b1[128:]], axis=1))
    b2_np = np.ascontiguousarray(b2[:, None])
    eye_np = np.eye(128, dtype=np.float32)

    in_maps = []
    for core in cores:
        t0, t1 = core["t0"], core["t1"]
        xT_c = np.zeros((D_IN, t_pad), dtype=np.float32)
        xT_c[:, : t1 - t0] = x[t0:t1].T
        in_maps.append(
            {
                "xT": xT_c,
                "w1": W1,
                "w2": w2_np,
                "b1": b1_np,
                "b2": b2_np,
                "eye": eye_np,
                "gidx": core["gidx"],
                "invc": core["invc"],
            }
        )

    res = run_bass_kernel_spmd(nc, in_maps, list(range(N_CORES)))

    out_full = np.zeros((n_seg, D_OUT), dtype=np.float32)
    for c, core in enumerate(cores):
        s0, s1 = core["s0"], core["s1"]
        if s1 > s0:
            out_full[s0:s1] = res.results[c]["out"][core["slot_of"]]
    return out_full
